# revision 36
# baseline (speedup 1.0000x reference)
"""Binarized conv2d kernel for Trainium2, SPMD over 8 NeuronCores.

Math (forward-value equivalent of the reference):
    real_w  = sum_k RV[k] * weights[k]          # [256,256,3,3], exact fp32 on DVE
    scale   = mean(|real_w|, axis=(1,2,3))      # per out-channel
    out     = conv2d(sign(x), sign(real_w), pad=1) * (scale * alpha)

sign(x) and sign(real_w) are {-1,0,+1} which are exact in fp8e4, so the conv
is computed with fp8 DoubleRow matmuls (exact integer accumulation in fp32
PSUM) and the per-channel scale*alpha is applied on PSUM evacuation.

Sharding: data-parallel over batch, 4 images per core; weights/RV/alpha
replicated. No collectives.

Schedule (iteration 2):
  - Weights-half-0 DMAs go FIRST on the sync ring (they gate the PE
    pipeline: mix -> sign -> transpose -> conv). x0 is split in two row
    slabs so conv(0,0) ptile A can start before all of x0 has landed.
  - sign(x_{b+1}) is emitted between ptile A and ptile B of conv(b,1) so
    it never blocks PSUM evacuation in the ACT queue (it used to stall
    the PE ~5us per image boundary waiting for PSUM WAR).
  - Plane rows are 57 wide (one shared guard column between rows instead
    of two): matmul free dim drops 464 -> 456 (-1.7% PE time).
  - Output is written fp16 (tolerance 2e-2, fp16 adds ~5e-4): halves
    output DMA bytes; host converts back to fp32.
"""

import numpy as np
from contextlib import ExitStack

import concourse.bass as bass
import concourse.bacc as bacc
import concourse.tile as tile
from concourse import mybir
from concourse.bass_utils import run_bass_kernel_spmd
from concourse.masks import make_identity

# Problem shapes (hardcoded per contract)
B, C, H, W = 32, 256, 56, 56
K, KS = 4, 3
NCORES = 8
BL = B // NCORES            # images per core

PW = W + 1                  # row pitch 57: one guard col shared between rows
PLANE = (H + 2) * PW        # 58 rows x 57
PL = 3312                   # plane stride (>= GO+PLANE+1, multiple of 16)
GO = 1                      # guard offset: plane data starts at elem 1
RPC = 8                     # rows per chunk
CHUNK = RPC * PW            # 456 elems per matmul (one PSUM bank)
NCHUNK = H // RPC           # 7 chunks: psum tile A gets 4, tile B gets 3
PT_CHUNKS = (4, 3)
CIH = C // 128              # 2 ci halves
COH = C // 128              # 2 co halves
TAPS = KS * KS              # 9
ROWSA = 34                  # x rows in slab A of image 0 (ptile A reads rows
                            # <=32 plus a 1-elem dx overrun; 34 keeps a full
                            # row of margin vs interval-rounding in dep tracking

F32 = mybir.dt.float32
F16 = mybir.dt.float16
FP8 = mybir.dt.float8e4

_cache = {}


def _build():
    act_dt = FP8
    nc = bacc.Bacc("TRN2", target_bir_lowering=False, debug=False,
                   num_devices=NCORES)
    x_d = nc.dram_tensor("x", [BL, C, H, W], F32, kind="ExternalInput")
    w_d = nc.dram_tensor("weights", [K, C, C, KS, KS], F32, kind="ExternalInput")
    rv_d = nc.dram_tensor("RV", [K + 1], F32, kind="ExternalInput")
    al_d = nc.dram_tensor("alpha", [C, 1, 1], F32, kind="ExternalInput")
    o_d = nc.dram_tensor("out", [BL, C, H, W], F16, kind="ExternalOutput")

    with tile.TileContext(nc) as tc, ExitStack() as ctx:
        consts = ctx.enter_context(tc.tile_pool(name="consts", bufs=1))
        wstage = ctx.enter_context(tc.tile_pool(name="wstage", bufs=8))
        wwork = ctx.enter_context(tc.tile_pool(name="wwork", bufs=2))
        xin = ctx.enter_context(tc.tile_pool(name="xin", bufs=4))
        xpads = ctx.enter_context(tc.tile_pool(name="xpads", bufs=1))
        outp = ctx.enter_context(tc.tile_pool(name="outp", bufs=2))

        # --- tiny constant loads on the ACT HWDGE ring (keeps the sync
        # ring free for the big weight DMAs) -------------------------------
        rv = consts.tile([128, K], F32, tag="rv")
        rv_src = bass.AP(tensor=rv_d.ap().tensor, offset=0,
                         ap=[[0, 128], [1, K]])
        nc.scalar.dma_start(out=rv, in_=rv_src)
        alpha_sb = []
        for h in range(COH):
            t = consts.tile([128, 1], F32, tag=f"alpha{h}")
            nc.scalar.dma_start(out=t,
                                in_=al_d.ap()[h * 128:(h + 1) * 128, 0, :])
            alpha_sb.append(t)

        # Padded planes: zero only the pad borders on DVE (tiny strided
        # memsets — the interior is fully overwritten by sign(x) each image
        # and pads are never written again).
        # Layout: data row r (0..55) at plane row r+1; each 57-elem row is
        # [guard_col | 56 data]; the guard doubles as left pad of its row
        # and right pad of the previous row. Rows 0 and 57 are full pads.
        xpad = []
        for i in range(3):
            t = xpads.tile([128, CIH, PL], act_dt, tag=f"xpad{i}",
                           name=f"xpad{i}")
            for s in range(CIH):
                pl = t[:, s, :]
                # guard elem + top pad row + row-1 guard col
                nc.vector.memset(pl[:, 0:GO + PW + 1], 0.0)
                # guard col of data rows 2..56
                nc.vector.memset(
                    pl[:, GO + 2 * PW:GO + 2 * PW + 55 * PW].rearrange(
                        "p (r c) -> p r c", c=PW)[:, :, 0:1], 0.0)
                # bottom pad row + trailing guard
                nc.vector.memset(pl[:, GO + 57 * PW:PL], 0.0)
            xpad.append(t)
        ident = consts.tile([128, 128], act_dt, tag="ident")
        make_identity(nc, ident)

        wT = consts.tile([128, TAPS, COH, CIH, 128], act_dt, tag="wT")
        scale_alpha = [consts.tile([128, 1], F32, tag=f"sa{h}", name=f"sa{h}")
                       for h in range(COH)]

        # --- weight prep for one co-half: DMA, mix, sign -------------------
        # 8 DMAs per half, one per (ci-half, k), split across BOTH HWDGE
        # rings (ci-half 0 on the sync ring, ci-half 1 on the scalar ring)
        # so the two queues transfer concurrently. The mix is likewise
        # split: ci-half 0 on DVE, ci-half 1 on GpSimd, halving the serial
        # mix chain. DMA / mix / sign emission are separate so each
        # engine's in-order queue can be sequenced explicitly.
        HCI = C // CIH * TAPS  # 1152 mix columns per ci-half
        wmixes = {}

        def prep_dma_ci(h, ci, eng):
            wks = []
            for k in range(K):
                wk = wstage.tile([128, HCI], F32, tag=f"wsb{ci}", bufs=4,
                                 name="wk")
                wks.append(wk)
                eng.dma_start(
                    out=wk,
                    in_=w_d.ap()[k, h * 128:(h + 1) * 128,
                                 ci * (C // CIH):(ci + 1) * (C // CIH)]
                    .rearrange("p c a b -> p (c a b)"))
            return wks

        def mix_alloc(h):
            wmix = wwork.tile([128, C * TAPS], F32, tag="wmix", name="wmix")
            wmixes[h] = wmix
            return wmix

        def mix_both(h, wks_ci):
            # TensorScalarPtr is DVE-only in walrus codegen (Pool/GpSimd
            # reject it), so both ci-halves mix on DVE, in the tiles'
            # DMA-arrival order so the chain never waits needlessly.
            wmix = wmixes[h]
            for ci in range(CIH):
                for k in range(K):
                    dst = wmix[:, ci * HCI:(ci + 1) * HCI]
                    wk = wks_ci[ci][k]
                    nc.vector.scalar_tensor_tensor(
                        dst, wk, rv[:, k:k + 1], wk if k == 0 else dst,
                        mybir.AluOpType.mult,
                        mybir.AluOpType.bypass if k == 0 else
                        mybir.AluOpType.add)

        def sign_w(h, ci, ws):
            nc.scalar.sign(ws[:, ci * HCI:(ci + 1) * HCI],
                           wmixes[h][:, ci * HCI:(ci + 1) * HCI])

        def ws_alloc(h):
            return wwork.tile([128, C * TAPS], act_dt, tag=f"wsign{h}",
                              bufs=1, name=f"wsign{h}")

        # |real_w| row-sums + scale*alpha combine, on DVE (fills DMA-gated
        # bubbles between mix passes; ACT variants measured slower)
        def reduce_half(h):
            absum = consts.tile([128, 1], F32, tag=f"ab{h}", name=f"ab{h}")
            nc.vector.tensor_reduce(absum, wmixes[h], mybir.AxisListType.X,
                                    mybir.AluOpType.add,
                                    apply_absolute_value=True)
            nc.vector.scalar_tensor_tensor(
                scale_alpha[h], absum, 1.0 / (C * TAPS), alpha_sb[h],
                mybir.AluOpType.mult, mybir.AluOpType.mult)

        # --- transpose one co-half's sign-weights into wT -------------------
        # The 18 [128,128] transposes are staged across three PSUM regions
        # (ps0/ps1/tps) so the matmuls run back-to-back with only a few ACT
        # copies and no copy-WAR stalls.
        def transpose_half(h, wsgn, cpsum):
            wsv = wsgn.rearrange("p (ci t) -> p ci t", t=TAPS)
            if h == 0:
                # ps0 first (frees conv00's first psum tile early); ps1 copy
                # split in two so conv00's tap-0 weights land sooner
                stages = [("ps0", 4 * 512, 8, 9, 1), ("ps1", 3 * 512, 0, 6, 2),
                          ("tps", 512, 6, 8, 1)]
            else:
                # ps1 last: it must wait for the previous conv's ptile-B
                # evacuation, which lands latest
                stages = [("ps0", 4 * 512, 0, 2, 1), ("tps", 512, 2, 4, 1),
                          ("ps1", 3 * 512, 4, 9, 2)]
            for tag, width, ta, tb, ncopy in stages:
                tp = cpsum.tile([128, width], F32, tag=tag, bufs=1,
                                name=f"t{tag}")
                for i, (tap, ci) in enumerate(
                        [(t, c) for t in range(ta, tb) for c in range(CIH)]):
                    nc.tensor.matmul(
                        tp[:, i * 128:(i + 1) * 128],
                        wsv[:, ci * 128:(ci + 1) * 128, tap], ident,
                        start=True, stop=True)
                nt = tb - ta
                for ic in range(ncopy):
                    ca = ta + ic * nt // ncopy
                    cb = ta + (ic + 1) * nt // ncopy
                    o0 = (ca - ta) * CIH * 128
                    nc.scalar.copy(
                        wT[:, ca:cb, h, :, :],
                        tp[:, o0:o0 + (cb - ca) * CIH * 128].rearrange(
                            "p (t ci co) -> p t ci co", t=cb - ca, co=128))

        # --- load + sign x for image b --------------------------------------
        # ci-tile 0 rides the sync ring, ci-tile 1 the scalar ring (the two
        # queues each ramp independently, so splitting roughly doubles the
        # early-head bandwidth). The ACT sign is emitted separately so the
        # ACT queue order is controlled. xsb1 has 3 bufs: its DMA issues
        # sit in the ACT queue ahead of evacuations, so a WAR against a
        # sign that runs behind those evacuations would deadlock ACT.
        x_engs = (nc.sync, nc.sync)

        def load_s(b, s, r0=0, r1=H):
            xs = xin.tile([128, (r1 - r0) * W], F32, tag=f"xsb{s}",
                          bufs=2 + s, name="xsb")
            x_engs[s].dma_start(
                out=xs,
                in_=x_d.ap()[b, s * 128:(s + 1) * 128, r0:r1].rearrange(
                    "p a b -> p (a b)"))
            return xs

        def load(b, r0=0, r1=H):
            return [load_s(b, s, r0, r1) for s in range(CIH)]

        def sign(b, tiles, s_list=(0, 1), r0=0, r1=H):
            xp = xpad[b % 3]
            for s in s_list:
                dst = xp[:, s, GO:GO + PLANE].rearrange(
                    "p (y x) -> p y x", x=PW)[:, 1 + r0:1 + r1, 1:57]
                nc.scalar.sign(dst, tiles[s].rearrange(
                    "p (y x) -> p y x", x=W))

        # --- conv for one (image, co-half) ---------------------------------
        # sign_emit, if given, is called between the ptile-A and ptile-B
        # blocks: its ACT ops land between the two evacuations, where the
        # ACT queue has slack and nothing downstream blocks on them.
        def conv(b, h, cpsum, sign_emit=None, tail_split=False,
                 tap_order=None):
            xp = xpad[b % 3]
            osb = outp.tile([128, H * W], F16, tag="osb", name="osb")
            # consume taps in the order the transpose stages produce them
            # (h=0 stages tap 8 first, h=1 is naturally ordered)
            if tap_order is None:
                tap_order = ([8, 0, 1, 2, 3, 4, 5, 6, 7] if h == 0
                             else list(range(TAPS)))
            c0 = 0
            for t, nch in enumerate(PT_CHUNKS):
                ps = cpsum.tile([128, nch * 512], F32, tag=f"ps{t}", bufs=1,
                                name=f"ps{t}")
                for itap, tap in enumerate(tap_order):
                    dy, dx = tap // KS - 1, tap % KS - 1
                    lhsT = wT[:, tap, h, :, :]
                    for j in range(nch):
                        c = c0 + j
                        off = GO + (1 + RPC * c + dy) * PW + dx
                        o = ps[:, j * 512:j * 512 + CHUNK]
                        nc.tensor.matmul(
                            o, lhsT, xp[:, :, off:off + CHUNK],
                            start=(itap == 0), stop=(itap == TAPS - 1),
                            perf_mode=mybir.MatmulPerfMode.DoubleRow)
                # all PSUM evacuation on ACT (moving any of it to DVE lets
                # the static scheduler interleave it with the weight mix,
                # which measurably regresses). The last ptile of the kernel
                # is evacuated in two pieces so the post-matmul drain is
                # short.
                pieces = ((0, 2), (2, 3)) if (tail_split and t == 1) \
                    else ((0, nch),)
                for pa, pb in pieces:
                    src = ps.rearrange("p (c e) -> p c e", e=512)[
                        :, pa:pb, 0:CHUNK].rearrange(
                        "p c (r x) -> p c r x", x=PW)[:, :, :, 1:57]
                    dst = osb.rearrange("p (y x) -> p y x", x=W)[
                        :, (c0 + pa) * RPC:(c0 + pb) * RPC, :].rearrange(
                        "p (c r) x -> p c r x", r=RPC)
                    nc.scalar.activation(dst, src,
                                         mybir.ActivationFunctionType.Copy,
                                         bias=0.0, scale=scale_alpha[h])
                    # output DMA issued from gpsimd (idle): keeps the sync
                    # ring free for input DMAs and the ~0.6us descriptor
                    # generation off the ACT engine (which is the second
                    # busiest and serializes sign/evac work)
                    nc.gpsimd.dma_start(
                        out=o_d.ap()[b, h * 128:(h + 1) * 128,
                                     (c0 + pa) * RPC:(c0 + pb) * RPC,
                                     :].rearrange("p a b -> p (a b)"),
                        in_=osb[:, (c0 + pa) * RPC * W:(c0 + pb) * RPC * W])
                if t == 0 and sign_emit is not None:
                    sign_emit()
                c0 += nch

        # --- schedule --------------------------------------------------------
        # Ring orders (per-queue FIFO, queues ramp independently):
        #   sync:   w-h0-ci0 (4), x0-s0, x1-s0, w-h1-ci0 (4), x2-s0, x3-s0
        #   scalar: rv, alpha, w-h0-ci1 (4), x0-s1, x1-s1, w-h1-ci1 (4),
        #           x2-s1, x3-s1
        #   gpsimd: all output DMAs
        # Conv order (0,0), (1,0), (0,1), (1,1), (2,0).. lets image 1 run
        # on the already-loaded co-half-0 weights while the co-half-1
        # weight pipeline (DMA/mix/sign/transpose) completes — without this
        # the PE stalls ~5us waiting for w-h1 after conv(0,0).
        with tc.tile_pool(name="cpsum", bufs=1, space="PSUM") as cpsum:
            ws0 = ws_alloc(0)
            ws1 = ws_alloc(1)
            mix_alloc(0)
            wks00 = prep_dma_ci(0, 0, nc.sync)
            wks01 = prep_dma_ci(0, 1, nc.sync)
            # HAM warmup: fp32 matmuls gated on successive weight tiles so
            # the PE clock gate opens before the real transposes/convs
            # arrive and never sees a >3us idle gap. Results discarded.
            for wgate in (wks00[1], wks00[3], wks01[1], wks01[3]):
                wtp = cpsum.tile([128, 512], F32, tag="tps", bufs=1,
                                 name="warm")
                nc.tensor.matmul(wtp[:, 0:CHUNK], wgate[:, 0:128],
                                 wgate[:, 0:CHUNK], start=True, stop=True)
            # image 0 in two row slabs: conv(0,0) ptile A only needs rows
            # 0..32, so it can start ~4us before the rest of x0 lands
            xt0a = load(0, 0, ROWSA)
            xt0b = load(0, ROWSA, H)
            wks10 = prep_dma_ci(1, 0, nc.sync)
            wks11 = prep_dma_ci(1, 1, nc.sync)
            mix_both(0, (wks00, wks01))
            sign_w(0, 0, ws0)
            sign_w(0, 1, ws0)
            sign(0, xt0a, r0=0, r1=ROWSA)
            transpose_half(0, ws0, cpsum)
            sign(0, xt0b, r0=ROWSA, r1=H)
            reduce_half(0)
            xt1 = load(1)
            xt2 = load(2)
            xt3 = load(3)
            mix_alloc(1)
            # h1 mix/sign emitted *before* conv(0,0): their ACT slots land
            # ahead of the evacuations, so sign-w(h1) finishes the moment
            # its (DMA-gated) mix is done instead of queuing behind evacs —
            # that pulls the h1 transpose chain ~2us earlier
            mix_both(1, (wks10, wks11))
            sign_w(1, 0, ws1)
            sign_w(1, 1, ws1)
            conv(0, 0, cpsum)
            transpose_half(1, ws1, cpsum)
            reduce_half(1)
            conv(0, 1, cpsum, sign_emit=lambda: sign(1, xt1))
            conv(1, 0, cpsum)
            conv(1, 1, cpsum, sign_emit=lambda: sign(2, xt2))
            conv(2, 0, cpsum)
            conv(2, 1, cpsum, sign_emit=lambda: sign(3, xt3))
            conv(3, 0, cpsum)
            conv(3, 1, cpsum, tail_split=True)
    nc.compile()
    return nc


def _get_nc():
    if "nc" not in _cache:
        _cache["nc"] = _build()
    return _cache["nc"]


def run(inputs, trace=False):
    nc = _get_nc()
    x = np.ascontiguousarray(inputs["x"], dtype=np.float32)
    in_maps = [
        {
            "x": x[c * BL:(c + 1) * BL],
            "weights": np.ascontiguousarray(inputs["weights"], np.float32),
            "RV": np.ascontiguousarray(inputs["RV"], np.float32),
            "alpha": np.ascontiguousarray(inputs["alpha"], np.float32),
        }
        for c in range(NCORES)
    ]
    res = run_bass_kernel_spmd(nc, in_maps, core_ids=list(range(NCORES)),
                               trace=trace)
    out = np.concatenate([r["out"] for r in res.results], axis=0)
    return out.astype(np.float32), res


def kernel(**inputs) -> np.ndarray:
    out, _ = run(inputs, trace=False)
    return out


# revision 37
# speedup vs baseline: 1.0059x; 1.0059x over previous
"""Binarized conv2d kernel for Trainium2, SPMD over 8 NeuronCores.

Math (forward-value equivalent of the reference):
    real_w  = sum_k RV[k] * weights[k]          # [256,256,3,3], exact fp32 on DVE
    scale   = mean(|real_w|, axis=(1,2,3))      # per out-channel
    out     = conv2d(sign(x), sign(real_w), pad=1) * (scale * alpha)

sign(x) and sign(real_w) are {-1,0,+1} which are exact in fp8e4, so the conv
is computed with fp8 DoubleRow matmuls (exact integer accumulation in fp32
PSUM) and the per-channel scale*alpha is applied on PSUM evacuation.

Sharding: data-parallel over batch, 4 images per core; weights/RV/alpha
replicated. No collectives.

Schedule highlights (all engine queues are in-order; emission order is
scheduling):
  - All inputs ride the sync ring in strict priority order: w-h0 (8), x0
    (2 row-slabs x 2 ci-tiles), w-h1 (8), x1, x2, x3. Weights first: they
    gate the longest prep chain (mix -> sign -> transpose -> copy).
    Dual-ring input splits were tried and regressed: the ~430GB/s HBM cap
    is shared across queues, and DMA issues on the ACT engine delay the
    critical sign chain.
  - x0 is split in two row slabs so conv(0,0) ptile A starts before the
    rest of x0 lands; the wT copies are emitted between the two slab
    signs so they don't gate ptile A either.
  - h1's mix/sign are emitted before conv(0,0): their ACT slots sit ahead
    of the evacuations so sign-w(h1) runs the moment its DMA-gated mix
    finishes instead of queuing behind evacs.
  - sign(x_{b+1}) is emitted between ptile A and ptile B of conv(b,1) so
    it never blocks PSUM evacuation in the ACT queue (stalled the PE ~5us
    per image boundary otherwise). Three xpad planes make the sign's
    WAR reach back two images.
  - Plane rows are 57 wide (one shared guard column between rows instead
    of two): matmul free dim drops 464 -> 456 (-1.7% PE time).
  - Output is written fp16 (tolerance 2e-2, fp16 adds ~3e-4): halves
    output DMA bytes; host converts back to fp32. Output DMAs are issued
    from gpsimd (own queue, idle engine); the last ptile is evacuated in
    two pieces to shorten the post-matmul drain.
  - fp32 HAM-warmup matmuls gated on successive weight tiles keep the PE
    clock gate open through the DMA-bound head.
"""

import numpy as np
from contextlib import ExitStack

import concourse.bass as bass
import concourse.bacc as bacc
import concourse.tile as tile
from concourse import mybir
from concourse.bass_utils import run_bass_kernel_spmd
from concourse.masks import make_identity

# Problem shapes (hardcoded per contract)
B, C, H, W = 32, 256, 56, 56
K, KS = 4, 3
NCORES = 8
BL = B // NCORES            # images per core

PW = W + 1                  # row pitch 57: one guard col shared between rows
PLANE = (H + 2) * PW        # 58 rows x 57
PL = 3312                   # plane stride (>= GO+PLANE+1, multiple of 16)
GO = 1                      # guard offset: plane data starts at elem 1
RPC = 8                     # rows per chunk
CHUNK = RPC * PW            # 456 elems per matmul (one PSUM bank)
NCHUNK = H // RPC           # 7 chunks: psum tile A gets 4, tile B gets 3
PT_CHUNKS = (4, 3)
CIH = C // 128              # 2 ci halves
COH = C // 128              # 2 co halves
TAPS = KS * KS              # 9
ROWSA = 33                  # x rows in slab A of image 0 (ptile A needs 0..32)

F32 = mybir.dt.float32
F16 = mybir.dt.float16
FP8 = mybir.dt.float8e4

_cache = {}


def _build():
    act_dt = FP8
    nc = bacc.Bacc("TRN2", target_bir_lowering=False, debug=False,
                   num_devices=NCORES)
    x_d = nc.dram_tensor("x", [BL, C, H, W], F32, kind="ExternalInput")
    w_d = nc.dram_tensor("weights", [K, C, C, KS, KS], F32, kind="ExternalInput")
    rv_d = nc.dram_tensor("RV", [K + 1], F32, kind="ExternalInput")
    al_d = nc.dram_tensor("alpha", [C, 1, 1], F32, kind="ExternalInput")
    o_d = nc.dram_tensor("out", [BL, C, H, W], F16, kind="ExternalOutput")

    with tile.TileContext(nc) as tc, ExitStack() as ctx:
        consts = ctx.enter_context(tc.tile_pool(name="consts", bufs=1))
        wstage = ctx.enter_context(tc.tile_pool(name="wstage", bufs=8))
        wwork = ctx.enter_context(tc.tile_pool(name="wwork", bufs=2))
        xin = ctx.enter_context(tc.tile_pool(name="xin", bufs=4))
        xpads = ctx.enter_context(tc.tile_pool(name="xpads", bufs=1))
        outp = ctx.enter_context(tc.tile_pool(name="outp", bufs=2))

        # --- tiny constant loads on the ACT HWDGE ring (keeps the sync
        # ring free for the big weight DMAs) -------------------------------
        rv = consts.tile([128, K], F32, tag="rv")
        rv_src = bass.AP(tensor=rv_d.ap().tensor, offset=0,
                         ap=[[0, 128], [1, K]])
        nc.scalar.dma_start(out=rv, in_=rv_src)
        alpha_sb = []
        for h in range(COH):
            t = consts.tile([128, 1], F32, tag=f"alpha{h}")
            nc.scalar.dma_start(out=t,
                                in_=al_d.ap()[h * 128:(h + 1) * 128, 0, :])
            alpha_sb.append(t)

        # Padded planes: zero only the pad borders on DVE (tiny strided
        # memsets — the interior is fully overwritten by sign(x) each image
        # and pads are never written again).
        # Layout: data row r (0..55) at plane row r+1; each 57-elem row is
        # [guard_col | 56 data]; the guard doubles as left pad of its row
        # and right pad of the previous row. Rows 0 and 57 are full pads.
        xpad = []
        for i in range(3):
            t = xpads.tile([128, CIH, PL], act_dt, tag=f"xpad{i}",
                           name=f"xpad{i}")
            for s in range(CIH):
                pl = t[:, s, :]
                # guard elem + top pad row + row-1 guard col
                nc.vector.memset(pl[:, 0:GO + PW + 1], 0.0)
                # guard col of data rows 2..56
                nc.vector.memset(
                    pl[:, GO + 2 * PW:GO + 2 * PW + 55 * PW].rearrange(
                        "p (r c) -> p r c", c=PW)[:, :, 0:1], 0.0)
                # bottom pad row + trailing guard
                nc.vector.memset(pl[:, GO + 57 * PW:PL], 0.0)
            xpad.append(t)
        ident = consts.tile([128, 128], act_dt, tag="ident")
        make_identity(nc, ident)

        wT = consts.tile([128, TAPS, COH, CIH, 128], act_dt, tag="wT")
        scale_alpha = [consts.tile([128, 1], F32, tag=f"sa{h}", name=f"sa{h}")
                       for h in range(COH)]

        # --- weight prep for one co-half: DMA, mix, sign -------------------
        # 8 DMAs per half, one per (ci-half, k): 4608B contiguous runs keep
        # the DMA engine at full rate and the DVE mix of tile i runs while
        # tile i+1 transfers. DMA / mix / sign emission are separated so
        # each engine's in-order queue can be sequenced explicitly (a
        # data-gated weight op sitting early in the ACT queue would block
        # sign(x)/evacuations behind it).
        HCI = C // CIH * TAPS  # 1152 mix columns per ci-half
        wmixes = {}

        def prep_dma_ci(h, ci, eng):
            wks = []
            for k in range(K):
                wk = wstage.tile([128, HCI], F32, tag=f"wsb{ci}", bufs=4,
                                 name="wk")
                wks.append(wk)
                eng.dma_start(
                    out=wk,
                    in_=w_d.ap()[k, h * 128:(h + 1) * 128,
                                 ci * (C // CIH):(ci + 1) * (C // CIH)]
                    .rearrange("p c a b -> p (c a b)"))
            return wks

        def mix_alloc(h):
            wmix = wwork.tile([128, C * TAPS], F32, tag="wmix", name="wmix")
            wmixes[h] = wmix
            return wmix

        def mix_both(h, wks_ci):
            # TensorScalarPtr is DVE-only in walrus codegen (Pool/GpSimd
            # reject it), so both ci-halves mix on DVE, in the tiles'
            # DMA-arrival order so the chain never waits needlessly.
            wmix = wmixes[h]
            for ci in range(CIH):
                for k in range(K):
                    dst = wmix[:, ci * HCI:(ci + 1) * HCI]
                    wk = wks_ci[ci][k]
                    nc.vector.scalar_tensor_tensor(
                        dst, wk, rv[:, k:k + 1], wk if k == 0 else dst,
                        mybir.AluOpType.mult,
                        mybir.AluOpType.bypass if k == 0 else
                        mybir.AluOpType.add)

        def sign_w(h, ci, ws):
            nc.scalar.sign(ws[:, ci * HCI:(ci + 1) * HCI],
                           wmixes[h][:, ci * HCI:(ci + 1) * HCI])

        def ws_alloc(h):
            return wwork.tile([128, C * TAPS], act_dt, tag=f"wsign{h}",
                              bufs=1, name=f"wsign{h}")

        # |real_w| row-sums + scale*alpha combine, on DVE (fills DMA-gated
        # bubbles between mix passes; ACT variants measured slower)
        def reduce_half(h):
            absum = consts.tile([128, 1], F32, tag=f"ab{h}", name=f"ab{h}")
            nc.vector.tensor_reduce(absum, wmixes[h], mybir.AxisListType.X,
                                    mybir.AluOpType.add,
                                    apply_absolute_value=True)
            nc.vector.scalar_tensor_tensor(
                scale_alpha[h], absum, 1.0 / (C * TAPS), alpha_sb[h],
                mybir.AluOpType.mult, mybir.AluOpType.mult)

        # --- transpose one co-half's sign-weights into wT -------------------
        # The 18 [128,128] transposes are staged across three PSUM regions
        # (ps0/ps1/tps) so the matmuls run back-to-back with only a few ACT
        # copies and no copy-WAR stalls.
        def transpose_half(h, wsgn, cpsum):
            wsv = wsgn.rearrange("p (ci t) -> p ci t", t=TAPS)
            if h == 0:
                # ps0 first (frees conv00's first psum tile early); ps1 copy
                # split in two so conv00's tap-0 weights land sooner
                stages = [("ps0", 4 * 512, 8, 9, 1), ("ps1", 3 * 512, 0, 6, 2),
                          ("tps", 512, 6, 8, 1)]
            else:
                # ps1 last: it must wait for the previous conv's ptile-B
                # evacuation, which lands latest
                stages = [("ps0", 4 * 512, 0, 2, 1), ("tps", 512, 2, 4, 1),
                          ("ps1", 3 * 512, 4, 9, 2)]
            for tag, width, ta, tb, ncopy in stages:
                tp = cpsum.tile([128, width], F32, tag=tag, bufs=1,
                                name=f"t{tag}")
                for i, (tap, ci) in enumerate(
                        [(t, c) for t in range(ta, tb) for c in range(CIH)]):
                    nc.tensor.matmul(
                        tp[:, i * 128:(i + 1) * 128],
                        wsv[:, ci * 128:(ci + 1) * 128, tap], ident,
                        start=True, stop=True)
                nt = tb - ta
                for ic in range(ncopy):
                    ca = ta + ic * nt // ncopy
                    cb = ta + (ic + 1) * nt // ncopy
                    o0 = (ca - ta) * CIH * 128
                    nc.scalar.copy(
                        wT[:, ca:cb, h, :, :],
                        tp[:, o0:o0 + (cb - ca) * CIH * 128].rearrange(
                            "p (t ci co) -> p t ci co", t=cb - ca, co=128))

        # --- load + sign x for image b --------------------------------------
        # ci-tile 0 rides the sync ring, ci-tile 1 the scalar ring (the two
        # queues each ramp independently, so splitting roughly doubles the
        # early-head bandwidth). The ACT sign is emitted separately so the
        # ACT queue order is controlled. xsb1 has 3 bufs: its DMA issues
        # sit in the ACT queue ahead of evacuations, so a WAR against a
        # sign that runs behind those evacuations would deadlock ACT.
        x_engs = (nc.sync, nc.sync)

        def load_s(b, s, r0=0, r1=H):
            xs = xin.tile([128, (r1 - r0) * W], F32, tag=f"xsb{s}",
                          bufs=2 + s, name="xsb")
            x_engs[s].dma_start(
                out=xs,
                in_=x_d.ap()[b, s * 128:(s + 1) * 128, r0:r1].rearrange(
                    "p a b -> p (a b)"))
            return xs

        def load(b, r0=0, r1=H):
            return [load_s(b, s, r0, r1) for s in range(CIH)]

        def sign(b, tiles, s_list=(0, 1), r0=0, r1=H):
            xp = xpad[b % 3]
            for s in s_list:
                dst = xp[:, s, GO:GO + PLANE].rearrange(
                    "p (y x) -> p y x", x=PW)[:, 1 + r0:1 + r1, 1:57]
                nc.scalar.sign(dst, tiles[s].rearrange(
                    "p (y x) -> p y x", x=W))

        # --- conv for one (image, co-half) ---------------------------------
        # sign_emit, if given, is called between the ptile-A and ptile-B
        # blocks: its ACT ops land between the two evacuations, where the
        # ACT queue has slack and nothing downstream blocks on them.
        def conv(b, h, cpsum, sign_emit=None, tail_split=False,
                 tap_order=None):
            xp = xpad[b % 3]
            osb = outp.tile([128, H * W], F16, tag="osb", name="osb")
            # consume taps in the order the transpose stages produce them
            # (h=0 stages tap 8 first, h=1 is naturally ordered)
            if tap_order is None:
                tap_order = ([8, 0, 1, 2, 3, 4, 5, 6, 7] if h == 0
                             else list(range(TAPS)))
            c0 = 0
            for t, nch in enumerate(PT_CHUNKS):
                ps = cpsum.tile([128, nch * 512], F32, tag=f"ps{t}", bufs=1,
                                name=f"ps{t}")
                for itap, tap in enumerate(tap_order):
                    dy, dx = tap // KS - 1, tap % KS - 1
                    lhsT = wT[:, tap, h, :, :]
                    for j in range(nch):
                        c = c0 + j
                        off = GO + (1 + RPC * c + dy) * PW + dx
                        o = ps[:, j * 512:j * 512 + CHUNK]
                        nc.tensor.matmul(
                            o, lhsT, xp[:, :, off:off + CHUNK],
                            start=(itap == 0), stop=(itap == TAPS - 1),
                            perf_mode=mybir.MatmulPerfMode.DoubleRow)
                # all PSUM evacuation on ACT (moving any of it to DVE lets
                # the static scheduler interleave it with the weight mix,
                # which measurably regresses). The last ptile of the kernel
                # is evacuated in two pieces so the post-matmul drain is
                # short.
                pieces = ((0, 2), (2, 3)) if (tail_split and t == 1) \
                    else ((0, nch),)
                for pa, pb in pieces:
                    src = ps.rearrange("p (c e) -> p c e", e=512)[
                        :, pa:pb, 0:CHUNK].rearrange(
                        "p c (r x) -> p c r x", x=PW)[:, :, :, 1:57]
                    dst = osb.rearrange("p (y x) -> p y x", x=W)[
                        :, (c0 + pa) * RPC:(c0 + pb) * RPC, :].rearrange(
                        "p (c r) x -> p c r x", r=RPC)
                    nc.scalar.activation(dst, src,
                                         mybir.ActivationFunctionType.Copy,
                                         bias=0.0, scale=scale_alpha[h])
                    # output DMA issued from gpsimd (idle): keeps the sync
                    # ring free for input DMAs and the ~0.6us descriptor
                    # generation off the ACT engine (which is the second
                    # busiest and serializes sign/evac work)
                    nc.gpsimd.dma_start(
                        out=o_d.ap()[b, h * 128:(h + 1) * 128,
                                     (c0 + pa) * RPC:(c0 + pb) * RPC,
                                     :].rearrange("p a b -> p (a b)"),
                        in_=osb[:, (c0 + pa) * RPC * W:(c0 + pb) * RPC * W])
                if t == 0 and sign_emit is not None:
                    sign_emit()
                c0 += nch

        # --- schedule --------------------------------------------------------
        # Ring orders (per-queue FIFO, queues ramp independently):
        #   sync:   w-h0-ci0 (4), x0-s0, x1-s0, w-h1-ci0 (4), x2-s0, x3-s0
        #   scalar: rv, alpha, w-h0-ci1 (4), x0-s1, x1-s1, w-h1-ci1 (4),
        #           x2-s1, x3-s1
        #   gpsimd: all output DMAs
        # Conv order (0,0), (1,0), (0,1), (1,1), (2,0).. lets image 1 run
        # on the already-loaded co-half-0 weights while the co-half-1
        # weight pipeline (DMA/mix/sign/transpose) completes — without this
        # the PE stalls ~5us waiting for w-h1 after conv(0,0).
        with tc.tile_pool(name="cpsum", bufs=1, space="PSUM") as cpsum:
            ws0 = ws_alloc(0)
            ws1 = ws_alloc(1)
            mix_alloc(0)
            wks00 = prep_dma_ci(0, 0, nc.sync)
            wks01 = prep_dma_ci(0, 1, nc.sync)
            # HAM warmup: fp32 matmuls gated on successive weight tiles so
            # the PE clock gate opens before the real transposes/convs
            # arrive and never sees a >3us idle gap. Results discarded.
            for wgate in (wks00[1], wks00[3], wks01[1], wks01[3]):
                wtp = cpsum.tile([128, 512], F32, tag="tps", bufs=1,
                                 name="warm")
                nc.tensor.matmul(wtp[:, 0:CHUNK], wgate[:, 0:128],
                                 wgate[:, 0:CHUNK], start=True, stop=True)
            # image 0 in two row slabs: conv(0,0) ptile A only needs rows
            # 0..32, so it can start ~4us before the rest of x0 lands
            xt0a = load(0, 0, ROWSA)
            xt0b = load(0, ROWSA, H)
            wks10 = prep_dma_ci(1, 0, nc.sync)
            wks11 = prep_dma_ci(1, 1, nc.sync)
            mix_both(0, (wks00, wks01))
            sign_w(0, 0, ws0)
            sign_w(0, 1, ws0)
            sign(0, xt0a, r0=0, r1=ROWSA)
            transpose_half(0, ws0, cpsum)
            sign(0, xt0b, r0=ROWSA, r1=H)
            reduce_half(0)
            xt1 = load(1)
            xt2 = load(2)
            xt3 = load(3)
            mix_alloc(1)
            # h1 mix/sign emitted *before* conv(0,0): their ACT slots land
            # ahead of the evacuations, so sign-w(h1) finishes the moment
            # its (DMA-gated) mix is done instead of queuing behind evacs —
            # that pulls the h1 transpose chain ~2us earlier
            mix_both(1, (wks10, wks11))
            sign_w(1, 0, ws1)
            sign_w(1, 1, ws1)
            conv(0, 0, cpsum)
            transpose_half(1, ws1, cpsum)
            reduce_half(1)
            conv(0, 1, cpsum, sign_emit=lambda: sign(1, xt1))
            conv(1, 0, cpsum)
            conv(1, 1, cpsum, sign_emit=lambda: sign(2, xt2))
            conv(2, 0, cpsum)
            conv(2, 1, cpsum, sign_emit=lambda: sign(3, xt3))
            conv(3, 0, cpsum)
            conv(3, 1, cpsum, tail_split=True)
    nc.compile()
    return nc


def _get_nc():
    if "nc" not in _cache:
        _cache["nc"] = _build()
    return _cache["nc"]


def run(inputs, trace=False):
    nc = _get_nc()
    x = np.ascontiguousarray(inputs["x"], dtype=np.float32)
    in_maps = [
        {
            "x": x[c * BL:(c + 1) * BL],
            "weights": np.ascontiguousarray(inputs["weights"], np.float32),
            "RV": np.ascontiguousarray(inputs["RV"], np.float32),
            "alpha": np.ascontiguousarray(inputs["alpha"], np.float32),
        }
        for c in range(NCORES)
    ]
    res = run_bass_kernel_spmd(nc, in_maps, core_ids=list(range(NCORES)),
                               trace=trace)
    out = np.concatenate([r["out"] for r in res.results], axis=0)
    return out.astype(np.float32), res


def kernel(**inputs) -> np.ndarray:
    out, _ = run(inputs, trace=False)
    return out
